# revision 2
# baseline (speedup 1.0000x reference)
"""
Trainium2 Bass kernel for batched cross-attention:
  context[b] = softmax(q[b] @ tokens[b].T / sqrt(d)) @ tokens[b]
with x_latent (tokens) [16, 4096, 768] f32, prompts_latent (q) [16, 64, 768] f32.

Sharding: data-parallel over batch — 16 batches / 8 cores = 2 per core.

v2 (single-load + on-chip transpose): tokens are shipped ONCE per core in
d-major layout tt [768, 4096] bf16 (12.6 MB/core instead of 25 MB for two
layouts — the baseline was DMA-bound at ~92us).

Per n-tile t (128 tokens), with tt tiles [128d, 128n] as PE stationaries:
  - S^T[t] [128n, 64p]  = sum_c tt[c,t]^T @ qt[c]   (6 matmuls, 64-col streams)
  - tn[t]  [128n, 768d] = tt[:,t]^T                 (6 PE transposes, 128-col
    streams, sharing the stationary with the mm above them)
  - ACT: P^T[t] = exp(S^T[t] * scale) -> SBUF bf16 [128, 64]
  - DVE: tn[t] PSUM -> SBUF [128, 0:768] of a [128,769] tile whose col 768 is
    pre-seeded with 1.0 (ring slots are seeded once at kernel start)
  - mm2 (lag 2): O[64, 512] += P^T[t]^T tn[t][:,0:512]  and
    O2[64, 257] += P^T[t]^T tn[t][:,512:769] — col 256 of O2 accumulates the
    softmax row sums via the ones column: no separate sum instructions.
  - finalize: rec = 1/sums, out = [O | O2[:,0:256]] * rec (DVE), DMA out.

PE per tile: 14 instructions, ~1921 stream cols (~800ns @2.4GHz); weight
loads (~40ns each, deep pipeline) hide under streams. DMA ~36us/core runs
far ahead; DVE copies (~556ns/tile) and ACT exp (~313ns/tile) stay under the
PE period.
"""

import os
import sys

import numpy as np

for _p in ("/opt/trn_rl_repo", "/root/.axon_site/_ro/trn_rl_repo"):
    if os.path.isdir(_p) and _p not in sys.path:
        sys.path.append(_p)

import ml_dtypes
from contextlib import ExitStack

import concourse.bass as bass
import concourse.mybir as mybir
import concourse.tile as tile
from concourse import bacc
from concourse.bass_utils import run_bass_kernel_spmd
from concourse.masks import make_identity

BF16 = ml_dtypes.bfloat16

N_CORES = 8
B_TOTAL = 16
BPC = B_TOTAL // N_CORES
N = 4096
D = 768
P = 64
DC = D // 128   # 6 d-chunks
NT = N // 128   # 32 n-tiles per batch
GPT = 4         # n-tiles per DMA group
NG = NT // GPT  # 8 groups per batch
SCALE = float(D) ** -0.5
TN_BUFS = 6
TT_BUFS = 4

_cached_nc = None


def build_bass_program() -> bass.Bass:
    nc = bacc.Bacc("TRN2", target_bir_lowering=False, debug=False)
    qt = nc.declare_dram_parameter("qt", [BPC, D, P], mybir.dt.bfloat16, isOutput=False)
    tt = nc.declare_dram_parameter("tt", [BPC, D, N], mybir.dt.bfloat16, isOutput=False)
    out = nc.declare_dram_parameter("out", [BPC, P, D], mybir.dt.float32, isOutput=True)

    with tile.TileContext(nc) as tc, ExitStack() as ctx:
        singles = ctx.enter_context(tc.tile_pool(name="singles", bufs=1))
        qt_pool = ctx.enter_context(tc.tile_pool(name="qtp", bufs=2))
        tt_pool = ctx.enter_context(tc.tile_pool(name="ttp", bufs=TT_BUFS))
        tn_pool = ctx.enter_context(tc.tile_pool(name="tnp", bufs=TN_BUFS))
        pt_pool = ctx.enter_context(tc.tile_pool(name="ptp", bufs=6))
        o_pool = ctx.enter_context(tc.tile_pool(name="op", bufs=2))
        fin_pool = ctx.enter_context(tc.tile_pool(name="finp", bufs=2))
        ps = ctx.enter_context(tc.tile_pool(name="ps", bufs=2, space="PSUM"))

        ident = singles.tile([128, 128], mybir.dt.bfloat16)
        make_identity(nc, ident)

        # Pre-seed the ones column (col 768) of every tn ring slot once.
        for _ in range(TN_BUFS):
            t0 = tn_pool.tile([128, D + 1], mybir.dt.bfloat16, tag="tn", name="tn_seed")
            nc.vector.memset(t0[:, D:D + 1], 1.0)

        qt_ts = [None] * BPC
        o_ab = {}           # b -> (o_a, o_b2)
        group_tiles = {}    # (b, g) -> tt_g tile

        def load_qt(b):
            qt_ts[b] = qt_pool.tile([128, DC, P], mybir.dt.bfloat16, tag="qt", name="qt_t")
            nc.sync.dma_start(out=qt_ts[b], in_=qt[b].rearrange("(c p) m -> p c m", p=128))

        def load_group(b, g, split=False):
            tt_g = tt_pool.tile([128, DC, 512], mybir.dt.bfloat16, tag="ttg", name="tt_g")
            tt_r = tt[b].rearrange("(c p) n -> p c n", p=128)
            if split:
                # first group: per-chunk DMAs so the first stationary is ready
                # after ~128KB instead of the full 768KB burst
                for c in range(DC):
                    nc.sync.dma_start(out=tt_g[:, c, :], in_=tt_r[:, c, g * 512:(g + 1) * 512])
            else:
                nc.sync.dma_start(out=tt_g, in_=tt_r[:, :, g * 512:(g + 1) * 512])
            group_tiles[(b, g)] = tt_g

        def flat_group(i):
            return (i // NG, i % NG) if i < BPC * NG else None

        def mm2a(b2, t2, pt2, tn2):
            o_a, _ = o_ab[b2]
            nc.tensor.matmul(o_a, lhsT=pt2, rhs=tn2[:, 0:512],
                             start=(t2 == 0), stop=(t2 == NT - 1))

        def mm2b(b2, t2, pt2, tn2):
            _, o_b2 = o_ab[b2]
            nc.tensor.matmul(o_b2, lhsT=pt2, rhs=tn2[:, 512:D + 1],
                             start=(t2 == 0), stop=(t2 == NT - 1))
            if t2 == NT - 1:
                finalize(b2)

        def finalize(b):
            o_a, o_b2 = o_ab[b]
            rec = fin_pool.tile([P, 1], mybir.dt.float32, tag="rec", name="rec")
            nc.vector.reciprocal(rec, o_b2[:, 256:257])
            o_sb = o_pool.tile([P, D], mybir.dt.float32, tag="osb", name="o_sb")
            nc.vector.tensor_scalar_mul(o_sb[:, 0:512], o_a, rec)
            nc.vector.tensor_scalar_mul(o_sb[:, 512:D], o_b2[:, 0:256], rec)
            nc.sync.dma_start(out=out[b], in_=o_sb)

        # prologue
        load_qt(0)
        load_group(0, 0, split=True)
        load_group(0, 1)
        load_group(0, 2)

        pend = []
        for idx in range(BPC * NT):
            b, t = divmod(idx, NT)
            g, j = divmod(t, GPT)
            if t == 0 and b not in o_ab:
                o_a = ps.tile([P, 512], mybir.dt.float32, tag="o_a", name="o_a")
                o_b2 = ps.tile([P, 257], mybir.dt.float32, tag="o_b", name="o_b2")
                o_ab[b] = (o_a, o_b2)
            if j == 0:
                nxt = flat_group(idx // GPT + 3)
                if nxt is not None:
                    load_group(*nxt)
                if g == NG - 2 and b + 1 < BPC:
                    load_qt(b + 1)
            tt_g = group_tiles[(b, g)]
            qt_t = qt_ts[b]

            st_ps = ps.tile([128, P], mybir.dt.float32, tag="st", name="st_ps")
            tn_ps = ps.tile([128, D], mybir.dt.bfloat16, tag="tnps", name="tn_ps")
            mm2 = pend.pop(0) if len(pend) == 2 else None
            for c in range(DC):
                stat = tt_g[:, c, j * 128:(j + 1) * 128]
                nc.tensor.transpose(tn_ps[:, c * 128:(c + 1) * 128], stat, ident)
                nc.tensor.matmul(st_ps, lhsT=stat, rhs=qt_t[:, c, :],
                                 start=(c == 0), stop=(c == DC - 1))
                if mm2 is not None:
                    if c == 1:
                        mm2a(*mm2)
                    elif c == 4:
                        mm2b(*mm2)
            pt = pt_pool.tile([128, P], mybir.dt.bfloat16, tag="pt", name="pt")
            nc.scalar.activation(out=pt, in_=st_ps,
                                 func=mybir.ActivationFunctionType.Exp, scale=SCALE)
            tn_sb = tn_pool.tile([128, D + 1], mybir.dt.bfloat16, tag="tn", name="tn_sb")
            nc.vector.tensor_copy(tn_sb[:, 0:D], tn_ps)
            pend.append((b, t, pt, tn_sb))

        for b2, t2, pt2, tn2 in pend:
            mm2a(b2, t2, pt2, tn2)
            mm2b(b2, t2, pt2, tn2)

    nc.compile()
    return nc


def _get_nc() -> bass.Bass:
    global _cached_nc
    if _cached_nc is None:
        _cached_nc = build_bass_program()
    return _cached_nc


def _make_in_maps(x_latent: np.ndarray, prompts_latent: np.ndarray):
    tt_h = np.ascontiguousarray(
        x_latent.astype(BF16).transpose(0, 2, 1))                  # [16, D, N]
    qt_h = np.ascontiguousarray(prompts_latent.astype(BF16).transpose(0, 2, 1))
    return [
        {
            "qt": qt_h[c * BPC:(c + 1) * BPC],
            "tt": tt_h[c * BPC:(c + 1) * BPC],
        }
        for c in range(N_CORES)
    ]


def run(x_latent: np.ndarray, prompts_latent: np.ndarray, trace: bool = False):
    """Run on all 8 cores; returns (output [16, 64, 768] f32, BassKernelResults)."""
    nc = _get_nc()
    in_maps = _make_in_maps(np.asarray(x_latent), np.asarray(prompts_latent))
    res = run_bass_kernel_spmd(nc, in_maps, list(range(N_CORES)), trace=trace)
    out = np.concatenate([np.asarray(r["out"]) for r in res.results], axis=0)
    return out.astype(np.float32), res


def kernel(x_latent: np.ndarray, prompts_latent: np.ndarray) -> np.ndarray:
    out, _ = run(x_latent, prompts_latent, trace=False)
    return out


# revision 3
# speedup vs baseline: 1.0338x; 1.0338x over previous
"""
Trainium2 Bass kernel for batched cross-attention:
  context[b] = softmax(q[b] @ tokens[b].T / sqrt(d)) @ tokens[b]
with x_latent (tokens) [16, 4096, 768] f32, prompts_latent (q) [16, 64, 768] f32.

Sharding: data-parallel over batch — 16 batches / 8 cores = 2 per core.

v3 (single-load + on-chip transpose + coarse dependency granularity):
tokens ship ONCE per core in d-major layout tt [768, 4096] bf16 (12.6 MB/core
vs 25 MB for the baseline's two layouts, which was DMA-bound at ~92us).

Per n-tile t (128 tokens), tt tiles [128d, 128n] are PE stationaries shared by
two instructions each:
  - S^T slice [128n, 64p] += tt[c,t]^T @ qt[c]     (6 matmuls, 64-col streams)
  - tn[t] [128n, 768d]     = tt[:,t]^T             (6 PE transposes, 128-col)
  - mm2 (lag 8 tiles): O[64,512] += P^T[t]^T tn[t][:,0:512],
    O2[64,257] += P^T[t]^T tn[t][:,512:769] — col 256 of O2 accumulates the
    softmax row sums through a pre-seeded ones column.

Cross-engine dependencies cost ~110ns of PE pipe stall per waiting
instruction (semaphore check), so they are batched coarse:
  - ONE tt DMA per 8-tile group ([128, 6, 1024], 1.5 MB)
  - ONE ACT exp per 8-tile group (S^T slices packed in one [128,512] bank)
  - ONE DVE copy per 2-tile pair (tn psum pair [128,1536] -> strided SBUF
    [128,1538] whose ones columns are pre-seeded once per ring slot)
All copies stay on DVE so PE waits hit one semaphore counter and dedupe.
"""

import os
import sys

import numpy as np

for _p in ("/opt/trn_rl_repo", "/root/.axon_site/_ro/trn_rl_repo"):
    if os.path.isdir(_p) and _p not in sys.path:
        sys.path.append(_p)

import ml_dtypes
from contextlib import ExitStack

import concourse.bass as bass
import concourse.mybir as mybir
import concourse.tile as tile
from concourse import bacc
from concourse.bass_utils import run_bass_kernel_spmd
from concourse.masks import make_identity

BF16 = ml_dtypes.bfloat16

N_CORES = 8
B_TOTAL = 16
BPC = B_TOTAL // N_CORES
N = 4096
D = 768
P = 64
DC = D // 128   # 6 d-chunks
NT = N // 128   # 32 n-tiles per batch
GPT = 8         # n-tiles per group
NG = NT // GPT  # 4 groups per batch
SCALE = float(D) ** -0.5
TN_BUFS = 6
TT_BUFS = 3

_cached_nc = None


def build_bass_program() -> bass.Bass:
    nc = bacc.Bacc("TRN2", target_bir_lowering=False, debug=False)
    qt = nc.declare_dram_parameter("qt", [BPC, D, P], mybir.dt.bfloat16, isOutput=False)
    tt = nc.declare_dram_parameter("tt", [BPC, D, N], mybir.dt.bfloat16, isOutput=False)
    out = nc.declare_dram_parameter("out", [BPC, P, D], mybir.dt.float32, isOutput=True)

    with tile.TileContext(nc) as tc, ExitStack() as ctx:
        singles = ctx.enter_context(tc.tile_pool(name="singles", bufs=1))
        qt_pool = ctx.enter_context(tc.tile_pool(name="qtp", bufs=2))
        tt_pool = ctx.enter_context(tc.tile_pool(name="ttp", bufs=TT_BUFS))
        tn_pool = ctx.enter_context(tc.tile_pool(name="tnp", bufs=TN_BUFS))
        pt_pool = ctx.enter_context(tc.tile_pool(name="ptp", bufs=2))
        o_pool = ctx.enter_context(tc.tile_pool(name="op", bufs=2))
        fin_pool = ctx.enter_context(tc.tile_pool(name="finp", bufs=2))
        ps = ctx.enter_context(tc.tile_pool(name="ps", bufs=2, space="PSUM"))

        ident = singles.tile([128, 128], mybir.dt.bfloat16)
        make_identity(nc, ident)

        # Pre-seed the ones columns (768 and 1537) of every tn ring slot once.
        for _ in range(TN_BUFS):
            t0 = tn_pool.tile([128, 2 * D + 2], mybir.dt.bfloat16, tag="tn", name="tn_seed")
            nc.vector.memset(t0[:, D:D + 1], 1.0)
            nc.vector.memset(t0[:, 2 * D + 1:2 * D + 2], 1.0)

        qt_ts = [None] * BPC
        o_ab = {}           # b -> (o_a, o_b2)
        group_tiles = {}    # (b, g) -> tt_g tile

        def load_qt(b):
            qt_ts[b] = qt_pool.tile([128, DC, P], mybir.dt.bfloat16, tag="qt", name="qt_t")
            nc.sync.dma_start(out=qt_ts[b], in_=qt[b].rearrange("(c p) m -> p c m", p=128))

        def load_group(b, g, split=False):
            tt_g = tt_pool.tile([128, DC, GPT * 128], mybir.dt.bfloat16, tag="ttg", name="tt_g")
            tt_r = tt[b].rearrange("(c p) n -> p c n", p=128)
            lo, hi = g * GPT * 128, (g + 1) * GPT * 128
            if split:
                for c in range(DC):
                    nc.sync.dma_start(out=tt_g[:, c, :], in_=tt_r[:, c, lo:hi])
            else:
                nc.sync.dma_start(out=tt_g, in_=tt_r[:, :, lo:hi])
            group_tiles[(b, g)] = tt_g

        def flat_group(i):
            return (i // NG, i % NG) if i < BPC * NG else None

        def ensure_o(b):
            if b not in o_ab:
                o_a = ps.tile([P, 512], mybir.dt.float32, tag="o_a", bufs=1, name="o_a")
                o_b2 = ps.tile([P, 257], mybir.dt.float32, tag="o_b", bufs=1, name="o_b2")
                o_ab[b] = (o_a, o_b2)

        def mm2a(b2, t2, pt2, tn2, half):
            ensure_o(b2)
            o_a, _ = o_ab[b2]
            base = half * (D + 1)
            nc.tensor.matmul(o_a, lhsT=pt2, rhs=tn2[:, base:base + 512],
                             start=(t2 == 0), stop=(t2 == NT - 1))

        def mm2b(b2, t2, pt2, tn2, half):
            _, o_b2 = o_ab[b2]
            base = half * (D + 1)
            nc.tensor.matmul(o_b2, lhsT=pt2, rhs=tn2[:, base + 512:base + D + 1],
                             start=(t2 == 0), stop=(t2 == NT - 1))
            if t2 == NT - 1:
                finalize(b2)

        def finalize(b):
            o_a, o_b2 = o_ab[b]
            rec = fin_pool.tile([P, 1], mybir.dt.float32, tag="rec", name="rec")
            nc.vector.reciprocal(rec, o_b2[:, 256:257])
            o_sb = o_pool.tile([P, D], mybir.dt.float32, tag="osb", name="o_sb")
            nc.vector.tensor_scalar_mul(o_sb[:, 0:512], o_a, rec)
            nc.vector.tensor_scalar_mul(o_sb[:, 512:D], o_b2[:, 0:256], rec)
            nc.sync.dma_start(out=out[b], in_=o_sb)
            del o_ab[b]

        # prologue
        load_qt(0)
        load_group(0, 0, split=True)
        load_group(0, 1)

        mm2_q = []      # per-tile mm2 descriptors (b, t, pt_slice, tn_sb, half)
        group_accum = []  # (pt_g, tiles) being built for current group
        st_g = None
        tn_ps = None
        tn_halves = []  # [(tn_sb, tiles...)]

        for idx in range(BPC * NT):
            b, t = divmod(idx, NT)
            g, j = divmod(t, GPT)
            if j == 0:
                nxt = flat_group(idx // GPT + 2)
                if nxt is not None:
                    load_group(*nxt)
                if b + 1 < BPC and g == NG - 1:
                    load_qt(b + 1)
                st_g = ps.tile([128, GPT * P], mybir.dt.float32, tag="st", name="st_g")
                group_accum = []
            if j % 2 == 0:
                tn_ps = ps.tile([128, 2 * D], mybir.dt.bfloat16, tag="tnps", name="tn_ps")
            tt_g = group_tiles[(b, g)]
            qt_t = qt_ts[b]

            half = j % 2
            for c in range(DC):
                stat = tt_g[:, c, j * 128:(j + 1) * 128]
                nc.tensor.transpose(
                    tn_ps[:, half * D + c * 128:half * D + (c + 1) * 128], stat, ident)
                nc.tensor.matmul(st_g[:, j * P:(j + 1) * P], lhsT=stat, rhs=qt_t[:, c, :],
                                 start=(c == 0), stop=(c == DC - 1))
                if mm2_q:
                    if c == 1:
                        mm2a(*mm2_q[0])
                    elif c == 4:
                        mm2b(*mm2_q.pop(0))
            group_accum.append((b, t))

            if half == 1:
                # pair complete: one strided DVE copy psum -> sbuf
                tn_sb = tn_pool.tile([128, 2 * D + 2], mybir.dt.bfloat16, tag="tn", name="tn_sb")
                nc.vector.tensor_copy(
                    tn_sb.rearrange("p (k x) -> p k x", k=2)[:, :, 0:D],
                    tn_ps.rearrange("p (k x) -> p k x", k=2),
                )
                tn_halves.append(tn_sb)

            if j == GPT - 1:
                # group complete: one exp for all 8 tiles
                pt_g = pt_pool.tile([128, GPT * P], mybir.dt.bfloat16, tag="pt", name="pt_g")
                nc.scalar.activation(out=pt_g, in_=st_g,
                                     func=mybir.ActivationFunctionType.Exp, scale=SCALE)
                for k, (b2, t2) in enumerate(group_accum):
                    mm2_q.append((b2, t2, pt_g[:, k * P:(k + 1) * P],
                                  tn_halves[k // 2], k % 2))
                tn_halves = []

        while mm2_q:
            mm2a(*mm2_q[0])
            mm2b(*mm2_q.pop(0))

    nc.compile()
    return nc


def _get_nc() -> bass.Bass:
    global _cached_nc
    if _cached_nc is None:
        _cached_nc = build_bass_program()
    return _cached_nc


def _make_in_maps(x_latent: np.ndarray, prompts_latent: np.ndarray):
    tt_h = np.ascontiguousarray(
        x_latent.astype(BF16).transpose(0, 2, 1))                  # [16, D, N]
    qt_h = np.ascontiguousarray(prompts_latent.astype(BF16).transpose(0, 2, 1))
    return [
        {
            "qt": qt_h[c * BPC:(c + 1) * BPC],
            "tt": tt_h[c * BPC:(c + 1) * BPC],
        }
        for c in range(N_CORES)
    ]


def run(x_latent: np.ndarray, prompts_latent: np.ndarray, trace: bool = False):
    """Run on all 8 cores; returns (output [16, 64, 768] f32, BassKernelResults)."""
    nc = _get_nc()
    in_maps = _make_in_maps(np.asarray(x_latent), np.asarray(prompts_latent))
    res = run_bass_kernel_spmd(nc, in_maps, list(range(N_CORES)), trace=trace)
    out = np.concatenate([np.asarray(r["out"]) for r in res.results], axis=0)
    return out.astype(np.float32), res


def kernel(x_latent: np.ndarray, prompts_latent: np.ndarray) -> np.ndarray:
    out, _ = run(x_latent, prompts_latent, trace=False)
    return out


# revision 6
# speedup vs baseline: 1.0366x; 1.0027x over previous
"""
Trainium2 Bass kernel for batched cross-attention:
  context[b] = softmax(q[b] @ tokens[b].T / sqrt(d)) @ tokens[b]
with x_latent (tokens) [16, 4096, 768] f32, prompts_latent (q) [16, 64, 768] f32.

Sharding: data-parallel over batch — 16 batches / 8 cores = 2 per core.

v4: single token load (d-major tt, 12.6 MB/core) + on-chip transpose, with the
PE pipeline kept free of semaphore-check stalls:

Per n-tile t (128 tokens), tt tiles [128d, 128n] are PE stationaries shared by
two instructions:
  - S^T slice [128n, 64p] += tt[c,t]^T @ qt[c]    (6 matmuls, 64-col streams)
  - tn[t] [128n, 768d]     = tt[:,t]^T            (6 PE transposes, 128-col)
  - mm2 (lag 4 tiles): O[64,512] += P^T^T tn[:,0:512],
    O2[64,257] += P^T^T tn[:,512:769] — col 256 of O2 accumulates the softmax
    row sums through a pre-seeded ones column in the SBUF tn tiles.

Stall avoidance (each semaphore check costs ~100ns and starves the weight-load
pipe for ~450ns if it lands between short streams):
  - ONE tt DMA per 8-tile group; ONE ACT exp per 4-tile half-group; ONE DVE
    copy per 2-tile pair (psum pair [128,1536] -> strided SBUF [128,1538]).
  - wait-carrying instructions (pair-start transposes) are emitted right after
    mm2a's 213ns stream so their checks hide under it.
  - the identity matrix ships from the host (make_identity needs the GpSimd
    library load, which otherwise delays the first transpose by ~7us).
  - the very first tt DMA is a 32KB slice so the PE starts ~2.5us in.
"""

import os
import sys

import numpy as np

for _p in ("/opt/trn_rl_repo", "/root/.axon_site/_ro/trn_rl_repo"):
    if os.path.isdir(_p) and _p not in sys.path:
        sys.path.append(_p)

import ml_dtypes
from contextlib import ExitStack

import concourse.bass as bass
import concourse.mybir as mybir
import concourse.tile as tile
from concourse import bacc
from concourse.bass_utils import run_bass_kernel_spmd

BF16 = ml_dtypes.bfloat16

N_CORES = 8
B_TOTAL = 16
BPC = B_TOTAL // N_CORES
N = 4096
D = 768
P = 64
DC = D // 128   # 6 d-chunks
NT = N // 128   # 32 n-tiles per batch
GPT = 8         # n-tiles per DMA/exp group
NG = NT // GPT  # 4 groups per batch
HG = 4          # tiles per exp half-group
SCALE = float(D) ** -0.5
TN_BUFS = 8
TT_BUFS = 3

_cached_nc = None


def build_bass_program() -> bass.Bass:
    nc = bacc.Bacc("TRN2", target_bir_lowering=False, debug=False)
    qt = nc.declare_dram_parameter("qt", [BPC, D, P], mybir.dt.bfloat16, isOutput=False)
    tt = nc.declare_dram_parameter("tt", [BPC, D, N], mybir.dt.bfloat16, isOutput=False)
    idm = nc.declare_dram_parameter("idm", [128, 128], mybir.dt.bfloat16, isOutput=False)
    out = nc.declare_dram_parameter("out", [BPC, P, D], mybir.dt.float32, isOutput=True)

    with tile.TileContext(nc) as tc, ExitStack() as ctx:
        singles = ctx.enter_context(tc.tile_pool(name="singles", bufs=1))
        qt_pool = ctx.enter_context(tc.tile_pool(name="qtp", bufs=2))
        tt_pool = ctx.enter_context(tc.tile_pool(name="ttp", bufs=TT_BUFS))
        tn_pool = ctx.enter_context(tc.tile_pool(name="tnp", bufs=TN_BUFS))
        pt_pool = ctx.enter_context(tc.tile_pool(name="ptp", bufs=4))
        o_pool = ctx.enter_context(tc.tile_pool(name="op", bufs=2))
        fin_pool = ctx.enter_context(tc.tile_pool(name="finp", bufs=2))
        ps = ctx.enter_context(tc.tile_pool(name="ps", bufs=2, space="PSUM"))

        ident = singles.tile([128, 128], mybir.dt.bfloat16)
        nc.sync.dma_start(out=ident, in_=idm[:, :])

        # Pre-seed the ones columns (768 and 1537) of every tn ring slot once.
        for _ in range(TN_BUFS):
            t0 = tn_pool.tile([128, 2 * D + 2], mybir.dt.bfloat16, tag="tn", name="tn_seed")
            nc.vector.memset(t0[:, D:D + 1], 1.0)
            nc.vector.memset(t0[:, 2 * D + 1:2 * D + 2], 1.0)

        qt_ts = [None] * BPC
        o_ab = {}           # b -> (o_a, o_b2)
        group_tiles = {}    # (b, g) -> tt_g tile

        def load_qt(b):
            qt_ts[b] = qt_pool.tile([128, DC, P], mybir.dt.bfloat16, tag="qt", name="qt_t")
            nc.sync.dma_start(out=qt_ts[b], in_=qt[b].rearrange("(c p) m -> p c m", p=128))

        def load_group(b, g, split=False):
            tt_g = tt_pool.tile([128, DC, GPT * 128], mybir.dt.bfloat16, tag="ttg", name="tt_g")
            tt_r = tt[b].rearrange("(c p) n -> p c n", p=128)
            lo, hi = g * GPT * 128, (g + 1) * GPT * 128
            if split:
                # head: tiny first slice so the first stationary lands fast
                nc.sync.dma_start(out=tt_g[:, 0, 0:128], in_=tt_r[:, 0, lo:lo + 128])
                nc.sync.dma_start(out=tt_g[:, 0, 128:GPT * 128], in_=tt_r[:, 0, lo + 128:hi])
                for c in range(1, DC):
                    nc.sync.dma_start(out=tt_g[:, c, :], in_=tt_r[:, c, lo:hi])
            else:
                nc.sync.dma_start(out=tt_g, in_=tt_r[:, :, lo:hi])
            group_tiles[(b, g)] = tt_g

        def flat_group(i):
            return (i // NG, i % NG) if i < BPC * NG else None

        def ensure_o(b):
            if b not in o_ab:
                o_a = ps.tile([P, 512], mybir.dt.float32, tag="o_a", bufs=1, name="o_a")
                o_b2 = ps.tile([P, 257], mybir.dt.float32, tag="o_b", bufs=1, name="o_b2")
                o_ab[b] = (o_a, o_b2)

        def mm2a(b2, t2, pt2, tn2, half):
            ensure_o(b2)
            o_a, _ = o_ab[b2]
            base = half * (D + 1)
            nc.tensor.matmul(o_a, lhsT=pt2, rhs=tn2[:, base:base + 512],
                             start=(t2 == 0), stop=(t2 == NT - 1))

        def mm2b(b2, t2, pt2, tn2, half):
            _, o_b2 = o_ab[b2]
            base = half * (D + 1)
            nc.tensor.matmul(o_b2, lhsT=pt2, rhs=tn2[:, base + 512:base + D + 1],
                             start=(t2 == 0), stop=(t2 == NT - 1))
            if t2 == NT - 1:
                finalize(b2)

        def finalize(b):
            o_a, o_b2 = o_ab[b]
            rec = fin_pool.tile([P, 1], mybir.dt.float32, tag="rec", name="rec")
            nc.vector.reciprocal(rec, o_b2[:, 256:257])
            o_sb = o_pool.tile([P, D], mybir.dt.float32, tag="osb", name="o_sb")
            nc.vector.tensor_scalar_mul(o_sb[:, 0:512], o_a, rec)
            nc.vector.tensor_scalar_mul(o_sb[:, 512:D], o_b2[:, 0:256], rec)
            nc.sync.dma_start(out=out[b], in_=o_sb)
            del o_ab[b]

        # prologue
        load_qt(0)
        load_group(0, 0, split=True)
        load_group(0, 1)

        mm2_q = []        # per-tile mm2 descriptors (b, t, pt_slice, tn_sb, half)
        st_g = None
        tn_ps = None
        tn_halves = []    # tn_sb tiles of the current half-group
        half_accum = []   # (b, t) of tiles in current half-group

        for idx in range(BPC * NT):
            b, t = divmod(idx, NT)
            g, j = divmod(t, GPT)
            if j == 0:
                nxt = flat_group(idx // GPT + 2)
                if nxt is not None:
                    load_group(*nxt)
                if b + 1 < BPC and g == NG - 1:
                    load_qt(b + 1)
                st_g = ps.tile([128, GPT * P], mybir.dt.float32, tag="st", name="st_g")
            if j % 2 == 0:
                tn_ps = ps.tile([128, 2 * D], mybir.dt.bfloat16, tag="tnps", name="tn_ps")
            tt_g = group_tiles[(b, g)]
            qt_t = qt_ts[b]
            half = j % 2

            # mm2a first: its 213ns stream hides the pair-start transpose's
            # semaphore checks (tn psum WAR + scheduler self-wait). Consumption
            # starts a half-group late so each pt_h exp is ~4 tiles old.
            mm2 = mm2_q.pop(0) if (mm2_q and idx >= GPT) else None
            if mm2 is not None:
                mm2a(*mm2)
            for c in range(DC):
                stat = tt_g[:, c, j * 128:(j + 1) * 128]
                nc.tensor.transpose(
                    tn_ps[:, half * D + c * 128:half * D + (c + 1) * 128], stat, ident)
                nc.tensor.matmul(st_g[:, j * P:(j + 1) * P], lhsT=stat, rhs=qt_t[:, c, :],
                                 start=(c == 0), stop=(c == DC - 1))
                if mm2 is not None and c == 2:
                    mm2b(*mm2)
            half_accum.append((b, t))

            if half == 1:
                tn_sb = tn_pool.tile([128, 2 * D + 2], mybir.dt.bfloat16, tag="tn", name="tn_sb")
                nc.vector.tensor_copy(
                    tn_sb.rearrange("p (k x) -> p k x", k=2)[:, :, 0:D],
                    tn_ps.rearrange("p (k x) -> p k x", k=2),
                )
                tn_halves.append(tn_sb)

            if j % HG == HG - 1:
                # half-group complete: one exp for 4 tiles
                h = (j // HG) % 2
                pt_h = pt_pool.tile([128, HG * P], mybir.dt.bfloat16, tag="pt", name="pt_h")
                nc.scalar.activation(out=pt_h, in_=st_g[:, h * HG * P:(h + 1) * HG * P],
                                     func=mybir.ActivationFunctionType.Exp, scale=SCALE)
                for k, (b2, t2) in enumerate(half_accum):
                    mm2_q.append((b2, t2, pt_h[:, k * P:(k + 1) * P],
                                  tn_halves[k // 2], k % 2))
                tn_halves = []
                half_accum = []

        while mm2_q:
            mm2 = mm2_q.pop(0)
            mm2a(*mm2)
            mm2b(*mm2)

    nc.compile()
    return nc


def _get_nc() -> bass.Bass:
    global _cached_nc
    if _cached_nc is None:
        _cached_nc = build_bass_program()
    return _cached_nc


def _make_in_maps(x_latent: np.ndarray, prompts_latent: np.ndarray):
    tt_h = np.ascontiguousarray(
        x_latent.astype(BF16).transpose(0, 2, 1))                  # [16, D, N]
    qt_h = np.ascontiguousarray(prompts_latent.astype(BF16).transpose(0, 2, 1))
    idm = np.eye(128, dtype=BF16)
    return [
        {
            "qt": qt_h[c * BPC:(c + 1) * BPC],
            "tt": tt_h[c * BPC:(c + 1) * BPC],
            "idm": idm,
        }
        for c in range(N_CORES)
    ]


def run(x_latent: np.ndarray, prompts_latent: np.ndarray, trace: bool = False):
    """Run on all 8 cores; returns (output [16, 64, 768] f32, BassKernelResults)."""
    nc = _get_nc()
    in_maps = _make_in_maps(np.asarray(x_latent), np.asarray(prompts_latent))
    res = run_bass_kernel_spmd(nc, in_maps, list(range(N_CORES)), trace=trace)
    out = np.concatenate([np.asarray(r["out"]) for r in res.results], axis=0)
    return out.astype(np.float32), res


def kernel(x_latent: np.ndarray, prompts_latent: np.ndarray) -> np.ndarray:
    out, _ = run(x_latent, prompts_latent, trace=False)
    return out


# revision 9
# speedup vs baseline: 1.2809x; 1.2357x over previous
"""
Trainium2 Bass kernel for batched cross-attention:
  context[b] = softmax(q[b] @ tokens[b].T / sqrt(d)) @ tokens[b]
with x_latent (tokens) [16, 4096, 768] f32, prompts_latent (q) [16, 64, 768] f32.

Sharding: data-parallel over batch — 16 batches / 8 cores = 2 per core.

v4: single token load (d-major tt, 12.6 MB/core) + on-chip transpose, with the
PE pipeline kept free of semaphore-check stalls:

Per n-tile t (128 tokens), tt tiles [128d, 128n] are PE stationaries shared by
two instructions:
  - S^T slice [128n, 64p] += tt[c,t]^T @ qt[c]    (6 matmuls, 64-col streams)
  - tn[t] [128n, 768d]     = tt[:,t]^T            (6 PE transposes, 128-col)
  - mm2 (lag 4 tiles): O[64,512] += P^T^T tn[:,0:512],
    O2[64,257] += P^T^T tn[:,512:769] — col 256 of O2 accumulates the softmax
    row sums through a pre-seeded ones column in the SBUF tn tiles.

Stall avoidance (each semaphore check costs ~100ns and starves the weight-load
pipe for ~450ns if it lands between short streams):
  - ONE tt DMA per 8-tile group; ONE ACT exp per 4-tile half-group; ONE DVE
    copy per 2-tile pair (psum pair [128,1536] -> strided SBUF [128,1538]).
  - wait-carrying instructions (pair-start transposes) are emitted right after
    mm2a's 213ns stream so their checks hide under it.
  - the identity matrix ships from the host (make_identity needs the GpSimd
    library load, which otherwise delays the first transpose by ~7us).
  - the very first tt DMA is a 32KB slice so the PE starts ~2.5us in.
"""

import os
import sys

import numpy as np

for _p in ("/opt/trn_rl_repo", "/root/.axon_site/_ro/trn_rl_repo"):
    if os.path.isdir(_p) and _p not in sys.path:
        sys.path.append(_p)

import ml_dtypes
from contextlib import ExitStack

import concourse.bass as bass
import concourse.mybir as mybir
import concourse.tile as tile
from concourse import bacc
from concourse.bass_utils import run_bass_kernel_spmd

BF16 = ml_dtypes.bfloat16

N_CORES = 8
B_TOTAL = 16
BPC = B_TOTAL // N_CORES
N = 4096
D = 768
P = 64
DC = D // 128   # 6 d-chunks
NT = N // 128   # 32 n-tiles per batch
GPT = 8         # n-tiles per DMA/exp group
NG = NT // GPT  # 4 groups per batch
HG = 4          # tiles per exp half-group
SCALE = float(D) ** -0.5
TN_BUFS = 18
TT_BUFS = 3

_cached_nc = None


def build_bass_program() -> bass.Bass:
    nc = bacc.Bacc("TRN2", target_bir_lowering=False, debug=False)
    qt = nc.declare_dram_parameter("qt", [BPC, D, P], mybir.dt.bfloat16, isOutput=False)
    tt = nc.declare_dram_parameter("tt", [BPC, D, N], mybir.dt.bfloat16, isOutput=False)
    idm = nc.declare_dram_parameter("idm", [128, 128], mybir.dt.bfloat16, isOutput=False)
    out = nc.declare_dram_parameter("out", [BPC, P, D], mybir.dt.float32, isOutput=True)

    with tile.TileContext(nc) as tc, ExitStack() as ctx:
        singles = ctx.enter_context(tc.tile_pool(name="singles", bufs=1))
        qt_pool = ctx.enter_context(tc.tile_pool(name="qtp", bufs=2))
        tt_pool = ctx.enter_context(tc.tile_pool(name="ttp", bufs=TT_BUFS))
        tn_pool = ctx.enter_context(tc.tile_pool(name="tnp", bufs=TN_BUFS))
        pt_pool = ctx.enter_context(tc.tile_pool(name="ptp", bufs=9))
        o_pool = ctx.enter_context(tc.tile_pool(name="op", bufs=2))
        fin_pool = ctx.enter_context(tc.tile_pool(name="finp", bufs=2))
        ps = ctx.enter_context(tc.tile_pool(name="ps", bufs=2, space="PSUM"))

        ident = singles.tile([128, 128], mybir.dt.bfloat16)
        nc.sync.dma_start(out=ident, in_=idm[:, :])

        # Pre-seed the ones columns (768 and 1537) of every tn ring slot once.
        for _ in range(TN_BUFS):
            t0 = tn_pool.tile([128, 2 * D + 2], mybir.dt.bfloat16, tag="tn", name="tn_seed")
            nc.vector.memset(t0[:, D:D + 1], 1.0)
            nc.vector.memset(t0[:, 2 * D + 1:2 * D + 2], 1.0)

        qt_ts = [None] * BPC
        o_ab = {}           # b -> (o_a, o_b2)
        group_tiles = {}    # (b, g) -> tt_g tile

        def load_qt(b):
            qt_ts[b] = qt_pool.tile([128, DC, P], mybir.dt.bfloat16, tag="qt", name="qt_t")
            nc.sync.dma_start(out=qt_ts[b], in_=qt[b].rearrange("(c p) m -> p c m", p=128))

        def load_group(b, g, split=False):
            tt_g = tt_pool.tile([128, DC, GPT * 128], mybir.dt.bfloat16, tag="ttg", name="tt_g")
            tt_r = tt[b].rearrange("(c p) n -> p c n", p=128)
            lo, hi = g * GPT * 128, (g + 1) * GPT * 128
            if split:
                # head: tiny first slice so the first stationary lands fast
                nc.sync.dma_start(out=tt_g[:, 0, 0:128], in_=tt_r[:, 0, lo:lo + 128])
                nc.sync.dma_start(out=tt_g[:, 0, 128:GPT * 128], in_=tt_r[:, 0, lo + 128:hi])
                for c in range(1, DC):
                    nc.sync.dma_start(out=tt_g[:, c, :], in_=tt_r[:, c, lo:hi])
            else:
                nc.sync.dma_start(out=tt_g, in_=tt_r[:, :, lo:hi])
            group_tiles[(b, g)] = tt_g

        def flat_group(i):
            return (i // NG, i % NG) if i < BPC * NG else None

        def ensure_o(b):
            if b not in o_ab:
                o_a = ps.tile([P, 512], mybir.dt.float32, tag="o_a", bufs=1, name="o_a")
                o_b2 = ps.tile([P, 257], mybir.dt.float32, tag="o_b", bufs=1, name="o_b2")
                o_ab[b] = (o_a, o_b2)

        def mm2a(b2, t2, pt2, tn2, half):
            ensure_o(b2)
            o_a, _ = o_ab[b2]
            base = half * (D + 1)
            nc.tensor.matmul(o_a, lhsT=pt2, rhs=tn2[:, base:base + 512],
                             start=(t2 == 0), stop=(t2 == NT - 1))

        def mm2b(b2, t2, pt2, tn2, half):
            _, o_b2 = o_ab[b2]
            base = half * (D + 1)
            nc.tensor.matmul(o_b2, lhsT=pt2, rhs=tn2[:, base + 512:base + D + 1],
                             start=(t2 == 0), stop=(t2 == NT - 1))
            if t2 == NT - 1:
                finalize(b2)

        def finalize(b):
            o_a, o_b2 = o_ab[b]
            rec = fin_pool.tile([P, 1], mybir.dt.float32, tag="rec", name="rec")
            nc.vector.reciprocal(rec, o_b2[:, 256:257])
            o_sb = o_pool.tile([P, D], mybir.dt.float32, tag="osb", name="o_sb")
            nc.vector.tensor_scalar_mul(o_sb[:, 0:512], o_a, rec)
            nc.vector.tensor_scalar_mul(o_sb[:, 512:D], o_b2[:, 0:256], rec)
            nc.sync.dma_start(out=out[b], in_=o_sb)
            del o_ab[b]

        # prologue
        load_qt(0)
        load_group(0, 0, split=True)
        load_group(0, 1)

        mm2_q = []        # per-tile mm2 descriptors (b, t, pt_slice, tn_sb, half)
        st_g = None
        tn_ps = None
        tn_halves = []    # tn_sb tiles of the current half-group
        half_accum = []   # (b, t) of tiles in current half-group

        for b in range(BPC):
            # Phase 1: all transposes + S^T matmuls for batch b (wait-light).
            for t in range(NT):
                idx = b * NT + t
                g, j = divmod(t, GPT)
                if j == 0:
                    nxt = flat_group(idx // GPT + 2)
                    if nxt is not None:
                        load_group(*nxt)
                    if b + 1 < BPC and g == NG - 1:
                        load_qt(b + 1)
                    st_g = ps.tile([128, GPT * P], mybir.dt.float32, tag="st", name="st_g")
                if j % 2 == 0:
                    tn_ps = ps.tile([128, 2 * D], mybir.dt.bfloat16, tag="tnps", name="tn_ps")
                tt_g = group_tiles[(b, g)]
                qt_t = qt_ts[b]
                half = j % 2

                for c in range(DC):
                    stat = tt_g[:, c, j * 128:(j + 1) * 128]
                    nc.tensor.transpose(
                        tn_ps[:, half * D + c * 128:half * D + (c + 1) * 128], stat, ident)
                    nc.tensor.matmul(st_g[:, j * P:(j + 1) * P], lhsT=stat, rhs=qt_t[:, c, :],
                                     start=(c == 0), stop=(c == DC - 1))
                half_accum.append((b, t))

                if half == 1:
                    tn_sb = tn_pool.tile([128, 2 * D + 2], mybir.dt.bfloat16, tag="tn", name="tn_sb")
                    nc.vector.tensor_copy(
                        tn_sb.rearrange("p (k x) -> p k x", k=2)[:, :, 0:D],
                        tn_ps.rearrange("p (k x) -> p k x", k=2),
                    )
                    tn_halves.append(tn_sb)

                if j % HG == HG - 1:
                    # half-group complete: one exp for 4 tiles
                    h = (j // HG) % 2
                    pt_h = pt_pool.tile([128, HG * P], mybir.dt.bfloat16, tag="pt", name="pt_h")
                    nc.scalar.activation(out=pt_h, in_=st_g[:, h * HG * P:(h + 1) * HG * P],
                                         func=mybir.ActivationFunctionType.Exp, scale=SCALE)
                    for k, (b2, t2) in enumerate(half_accum):
                        mm2_q.append((b2, t2, pt_h[:, k * P:(k + 1) * P],
                                      tn_halves[k // 2], k % 2))
                    tn_halves = []
                    half_accum = []

            # Phase 2: all mm2 pairs back-to-back; their semaphore checks hide
            # under the 213/107-col streams (measured clean at ~330ns/pair).
            while mm2_q:
                mm2 = mm2_q.pop(0)
                mm2a(*mm2)
                mm2b(*mm2)

    nc.compile()
    return nc


def _get_nc() -> bass.Bass:
    global _cached_nc
    if _cached_nc is None:
        _cached_nc = build_bass_program()
    return _cached_nc


def _make_in_maps(x_latent: np.ndarray, prompts_latent: np.ndarray):
    tt_h = np.ascontiguousarray(
        x_latent.astype(BF16).transpose(0, 2, 1))                  # [16, D, N]
    qt_h = np.ascontiguousarray(prompts_latent.astype(BF16).transpose(0, 2, 1))
    idm = np.eye(128, dtype=BF16)
    return [
        {
            "qt": qt_h[c * BPC:(c + 1) * BPC],
            "tt": tt_h[c * BPC:(c + 1) * BPC],
            "idm": idm,
        }
        for c in range(N_CORES)
    ]


def run(x_latent: np.ndarray, prompts_latent: np.ndarray, trace: bool = False):
    """Run on all 8 cores; returns (output [16, 64, 768] f32, BassKernelResults)."""
    nc = _get_nc()
    in_maps = _make_in_maps(np.asarray(x_latent), np.asarray(prompts_latent))
    res = run_bass_kernel_spmd(nc, in_maps, list(range(N_CORES)), trace=trace)
    out = np.concatenate([np.asarray(r["out"]) for r in res.results], axis=0)
    return out.astype(np.float32), res


def kernel(x_latent: np.ndarray, prompts_latent: np.ndarray) -> np.ndarray:
    out, _ = run(x_latent, prompts_latent, trace=False)
    return out
